# revision 6
# baseline (speedup 1.0000x reference)
"""Trainium2 Bass kernel for nn_GeneralAttn (multi-head attention with
structural attention bias + padding mask), data-parallel over batch B=8
across 8 NeuronCores.

v2 design notes (per core, one batch element):
  - Host pre-fuses pad_mask into attn_bias (bias + log(mask)) and casts to
    bf16: halves HBM traffic and removes the on-device mask multiply.
  - All matmuls run in bf16 (f32 PSUM accumulation). Weights/x are cast
    on-device (gpsimd) and transposed with the DMA XBAR transpose
    (dma_start_transpose) -- no PE transposes anywhere.
  - S = Q^T K + fused_bias per (128-query-block, head): PE matmul into a
    3-bank PSUM tile, DVE adds the bias (bf16 out), ACT exp with
    accum_out producing the softmax denominator in the same pass,
    gpsimd scales by the reciprocal (normalized P), XBAR transposes
    P -> P^T [k, q].
  - PV uses V as the stationary operand: O^T[d, q-block] accumulated over
    k-blocks, written directly in concat-head layout (catT). Head pairs
    land on PE partitions 0:64 / 64:128 (disjoint array row/col groups,
    so paired matmuls overlap in the array; copies are partition-aligned).
  - Out projection: catT^T @ Wo^T + bo via ones-row rank-1 matmul.
"""

import numpy as np
from contextlib import ExitStack

import ml_dtypes

import concourse.bass as bass
import concourse.bacc as bacc
import concourse.tile as tile
import concourse.mybir as mybir
from concourse.bass_utils import run_bass_kernel_spmd
from concourse._compat import with_exitstack

F32 = mybir.dt.float32
BF16 = mybir.dt.bfloat16
AF = mybir.ActivationFunctionType
OP = mybir.AluOpType

B = 8
NP = 1025
E = 512
H = 8
D = 64
N = NP - 1
NSUB = 9          # ceil(1025/128)
SEQ = NSUB * 128  # 1152
ESUB = 4          # 512/128
INV_SQRT_D = 1.0 / 8.0
NEG = -60000.0    # exp(NEG + anything bounded) == 0
# S-psum chunks along k: 512-col slices keep each matmul inside one PSUM bank
SCHUNKS = ((0, 512), (512, 512), (1024, 128))


@with_exitstack
def _attn_kernel(ctx: ExitStack, tc: tile.TileContext, aps: dict):
    nc = tc.nc

    # ---------------- persistent buffers ----------------
    persist = ctx.enter_context(tc.tile_pool(name="persist", bufs=1))
    QT = persist.tile([128, ESUB, SEQ], BF16, tag="QT")       # [dq, dsub, s]
    KT = persist.tile([128, ESUB, SEQ], BF16, tag="KT")
    V = persist.tile([128, NSUB, E], BF16, tag="V")           # [k, kblk, hd]
    catT = persist.tile([128, ESUB, SEQ], BF16, tag="catT")   # [hd, hsub, s]
    WTo = persist.tile([128, ESUB, ESUB, 128], BF16, tag="WTo")
    ones_bf = persist.tile([1, 128], BF16, tag="ones_bf")
    bo_row = persist.tile([1, E], BF16, tag="bo_row")
    bv_row = persist.tile([1, E], BF16, tag="bv_row")
    bqs = persist.tile([128, ESUB], F32, tag="bqs")
    bks = persist.tile([128, ESUB], F32, tag="bks")

    nc.gpsimd.memset(ones_bf[:], 1.0)

    # ---------------- setup (scoped pools; freed before main loop) -------
    with tc.tile_pool(name="setup", bufs=1) as setup, \
         tc.tile_pool(name="setup2", bufs=2) as setup2, \
         tc.tile_pool(name="pr_ps", bufs=2, space="PSUM") as pr_ps, \
         tc.tile_pool(name="pv_ps", bufs=2, space="PSUM") as pv_ps:

        # small vectors: bq/bk scaled by 1/sqrt(D) (fold into Q side)
        bsm = setup.tile([128, 2, ESUB], F32, tag="bsm")
        nc.sync.dma_start(out=bsm[:, 0], in_=aps["bq"].rearrange("(o p) -> p o", p=128))
        nc.sync.dma_start(out=bsm[:, 1], in_=aps["bk"].rearrange("(o p) -> p o", p=128))
        nc.gpsimd.tensor_scalar(bqs[:], bsm[:, 0], INV_SQRT_D, None, OP.mult)
        nc.gpsimd.tensor_copy(bks[:], bsm[:, 1])
        brow = setup.tile([1, 2, E], F32, tag="brow")
        nc.sync.dma_start(out=brow[:, 0], in_=aps["bo"].rearrange("(a e) -> a e", a=1))
        nc.sync.dma_start(out=brow[:, 1], in_=aps["bv"].rearrange("(a e) -> a e", a=1))
        nc.gpsimd.tensor_copy(bo_row[:], brow[:, 0])
        nc.gpsimd.tensor_copy(bv_row[:], brow[:, 1])

        # x: natural load, cast to bf16, XBAR-transpose per 128-row block
        xn = setup.tile([128, NSUB, E], F32, tag="xn")
        nc.gpsimd.memset(xn[:, 8, :], 0.0)
        nc.sync.dma_start(
            out=xn[:, 0:8, :], in_=aps["x"][0:1024, :].rearrange("(o p) f -> p o f", p=128)
        )
        nc.sync.dma_start(out=xn[0:1, 8, :], in_=aps["x"][1024:1025, :].rearrange("a f -> a f"))
        xbf = setup.tile([128, NSUB, E], BF16, tag="xbf")
        nc.gpsimd.tensor_copy(xbf[:], xn[:])
        # xT[:, s, e, :][p, c] = x^T[e*128+p, s*128+c]
        xT = setup.tile([128, NSUB, ESUB, 128], BF16, tag="xT")
        for s in range(NSUB):
            nc.scalar.dma_start_transpose(xT[:, s], xbf[:, s, :])

        # weights: load natural [dout, din], cast (Wq scaled), XBAR-transpose.
        # WTx[:, o, e, :][p, c] = W^T[e*128+p, o*128+c]  (o: dout blk, e: din blk)
        WTq = setup.tile([128, ESUB, ESUB, 128], BF16, tag="WTq")
        WTk = setup.tile([128, ESUB, ESUB, 128], BF16, tag="WTk")
        WTv = setup.tile([128, ESUB, ESUB, 128], BF16, tag="WTv")
        for wname, WTx, scale in (
            ("Wq", WTq, INV_SQRT_D), ("Wk", WTk, 1.0), ("Wv", WTv, 1.0), ("Wo", WTo, 1.0)
        ):
            wn = setup2.tile([128, ESUB, E], F32, tag="wn")
            nc.sync.dma_start(out=wn[:], in_=aps[wname].rearrange("(o p) f -> p o f", p=128))
            wb = setup2.tile([128, ESUB, E], BF16, tag="wb")
            nc.gpsimd.tensor_scalar(wb[:], wn[:], scale, None, OP.mult)
            for o in range(ESUB):
                nc.scalar.dma_start_transpose(WTx[:, o], wb[:, o, :])

        # Q^T / K^T projections: QT[:, dsub, s-block] = (W' x^T + b)
        # rhs spans up to 4 s-blocks via a 3D AP (512-col chunks, one PSUM bank)
        for WTx, QTx, brow_sb in ((WTq, QT, bqs), (WTk, KT, bks)):
            for dsub in range(ESUB):
                pr = pr_ps.tile([128, SEQ], F32, tag="pr")
                for e in range(ESUB):
                    for s0, sn in ((0, 4), (4, 4), (8, 1)):
                        nc.tensor.matmul(
                            pr[:, s0 * 128:(s0 + sn) * 128],
                            WTx[:, dsub, e, :],
                            xT[:, s0:s0 + sn, e, :],
                            start=(e == 0),
                            stop=(e == ESUB - 1),
                        )
                nc.vector.tensor_scalar(
                    QTx[:, dsub, :], pr[:], brow_sb[:, dsub:dsub + 1], None, OP.add
                )

        # V projection: V[:, s, :] = x W_v^T + bv   ([s, hd] natural)
        for s in range(NSUB):
            pv = pv_ps.tile([128, E], F32, tag="pv")
            nc.tensor.matmul(pv[:], ones_bf[:], bv_row[:], start=True, stop=False)
            for e in range(ESUB):
                nc.tensor.matmul(
                    pv[:],
                    xT[:, s, e, :],
                    WTv[:, 0:ESUB, e, :],
                    start=False,
                    stop=(e == ESUB - 1),
                    skip_group_check=True,
                )
            nc.vector.tensor_copy(V[:, s, :], pv[:])

    # ---------------- main loop (software-pipelined emission) ----------------
    # Per-head stage chain: bias DMA (SP) -> S matmul (PE) -> +bias (DVE)
    # -> exp+accum (ACT) -> rcp (DVE) -> scale (Pool) -> XBAR transpose (SP)
    # -> PV (PE) -> catT copy per pair (ACT) -> outproj per q-block (PE).
    # Engines execute their queues in order, so stages are emitted with a
    # skew (stage s of head t alongside stage s' of head t+k) to avoid
    # head-of-line blocking on cross-engine dependencies.
    with tc.tile_pool(name="bias_p", bufs=6) as bias_p, \
         tc.tile_pool(name="ssb_p", bufs=3) as ssb_p, \
         tc.tile_pool(name="p0_p", bufs=4) as p0_p, \
         tc.tile_pool(name="den_p", bufs=4) as den_p, \
         tc.tile_pool(name="pm_p", bufs=4) as pm_p, \
         tc.tile_pool(name="pt_p", bufs=4) as pt_p, \
         tc.tile_pool(name="osb_p", bufs=2) as osb_p, \
         tc.tile_pool(name="s_ps", bufs=2, space="PSUM") as s_ps, \
         tc.tile_pool(name="acc_ps", bufs=2, space="PSUM") as acc_ps:

        fused = aps["fused"]
        T = NSUB * H  # 72 heads total; t -> (qs, h) with qs-major order
        tiles = [dict() for _ in range(T)]

        def loc(t):
            qs, h = divmod(t, H)
            return qs, h, (128 if qs < 8 else 1), qs * 128, (h % 2) * 64, h // 2

        def em_bias(t):
            qs, h, rows, q0, hp0, hs = loc(t)
            bias_t = bias_p.tile([128, SEQ], BF16, tag="bias")
            nc.sync.dma_start(out=bias_t[0:rows, 0:NP], in_=fused[h, q0:q0 + rows, :])
            tiles[t]["bias"] = bias_t

        def em_s(t):
            qs, h, rows, q0, hp0, hs = loc(t)
            sp = s_ps.tile([128, SEQ], F32, tag="sps")
            qt = QT[hp0:hp0 + 64, hs, q0:q0 + 128]
            for c0, cm in SCHUNKS:
                nc.tensor.matmul(
                    sp[:, c0:c0 + cm], qt, KT[hp0:hp0 + 64, hs, c0:c0 + cm],
                    start=True, stop=True,
                )
            tiles[t]["sp"] = sp

        def em_add(t):
            sp = tiles[t].pop("sp")
            bias_t = tiles[t].pop("bias")
            s_sb = ssb_p.tile([128, SEQ], BF16, tag="ssb")
            nc.vector.tensor_tensor(s_sb[:, 0:NP], sp[:, 0:NP], bias_t[:, 0:NP], OP.add)
            tiles[t]["ssb"] = s_sb

        def em_exp(t):
            s_sb = tiles[t].pop("ssb")
            p0 = p0_p.tile([128, SEQ], BF16, tag="p0")
            denom = den_p.tile([128, 1], F32, tag="den")
            nc.scalar.activation(p0[:, 0:NP], s_sb[:, 0:NP], AF.Exp, accum_out=denom[:])
            tiles[t]["p0"] = p0
            tiles[t]["den"] = denom

        def em_rcp(t):
            denom = tiles[t].pop("den")
            rc = den_p.tile([128, 1], F32, tag="rc")
            nc.vector.reciprocal(rc[:], denom[:])
            tiles[t]["rc"] = rc

        def em_scale(t):
            p0 = tiles[t].pop("p0")
            rc = tiles[t].pop("rc")
            pm = pm_p.tile([128, SEQ], BF16, tag="pm")
            nc.vector.memset(pm[:, NP:SEQ], 0.0)
            nc.gpsimd.tensor_scalar(pm[:, 0:NP], p0[:, 0:NP], rc[:], None, OP.mult)
            tiles[t]["pm"] = pm

        def em_xbar(t):
            pm = tiles[t].pop("pm")
            pt = pt_p.tile([128, NSUB, 128], BF16, tag="pt")
            nc.sync.dma_start_transpose(pt[:], pm[:])
            tiles[t]["pt"] = pt

        def em_pv(t):
            qs, h, rows, q0, hp0, hs = loc(t)
            pt = tiles[t].pop("pt")
            if h % 2 == 0:
                ot = acc_ps.tile([128, E], F32, tag="acc")
                tiles[t]["ot"] = ot
            else:
                ot = tiles[t - 1]["ot"]
                tiles[t]["ot"] = ot
            for j in range(NSUB):
                nc.tensor.matmul(
                    ot[hp0:hp0 + 64, 0:128],
                    V[:, j, h * D:(h + 1) * D],
                    pt[:, j, :],
                    start=(j == 0),
                    stop=(j == NSUB - 1),
                    skip_group_check=True,
                )

        def em_cat(t):
            # t is the odd head of the pair; copy both halves at once
            qs, h, rows, q0, hp0, hs = loc(t)
            ot = tiles[t].pop("ot")
            tiles[t - 1].pop("ot", None)
            nc.scalar.copy(catT[:, hs, q0:q0 + 128], ot[:, 0:128])

        def em_outproj(t):
            qs, h, rows, q0, hp0, hs = loc(t)
            op = acc_ps.tile([128, E], F32, tag="acc")
            nc.tensor.matmul(op[:], ones_bf[:], bo_row[:], start=True, stop=False)
            for hd in range(ESUB):
                nc.tensor.matmul(
                    op[:],
                    catT[:, hd, q0:q0 + 128],
                    WTo[:, 0:ESUB, hd, :],
                    start=False,
                    stop=(hd == ESUB - 1),
                    skip_group_check=True,
                )
            tiles[t]["op"] = op

        def em_out(t):
            qs, h, rows, q0, hp0, hs = loc(t)
            op = tiles[t].pop("op")
            o_sb = osb_p.tile([128, E], F32, tag="osb")
            nc.scalar.copy(o_sb[0:rows, :], op[0:rows, :])
            nc.scalar.dma_start(out=aps["out"][q0:q0 + rows, :], in_=o_sb[0:rows, :])

        for t in range(T + 5):
            if t == 0:
                for u in range(min(4, T)):
                    em_bias(u)
                for u in range(min(2, T)):
                    em_s(u)
                em_add(0)
                em_exp(0)
            if t + 4 < T:
                em_bias(t + 4)
            if t + 2 < T:
                em_s(t + 2)
            if t + 1 < T:
                em_add(t + 1)
                em_exp(t + 1)
            if t < T:
                em_rcp(t)
                em_scale(t)
            if 0 <= t - 1 < T:
                em_xbar(t - 1)
            if 0 <= t - 2 < T:
                em_pv(t - 2)
                if (t - 2) % 2 == 1:
                    em_cat(t - 2)
            if 0 <= t - 3 < T and (t - 3) % H == H - 1:
                em_outproj(t - 3)
            if 0 <= t - 4 < T and (t - 4) % H == H - 1:
                em_out(t - 4)


_CACHE = {}


def _build(loop_factor: int = 1):
    key = ("nc", loop_factor)
    if key in _CACHE:
        return _CACHE[key]
    nc = bacc.Bacc("TRN2", num_devices=B)
    aps = {
        "x": nc.dram_tensor("x", [NP, E], F32, kind="ExternalInput").ap(),
        "fused": nc.dram_tensor("fused", [H, NP, NP], BF16, kind="ExternalInput").ap(),
    }
    for wname in ("Wq", "Wk", "Wv", "Wo"):
        aps[wname] = nc.dram_tensor(wname, [E, E], F32, kind="ExternalInput").ap()
    for bname in ("bq", "bk", "bv", "bo"):
        aps[bname] = nc.dram_tensor(bname, [E], F32, kind="ExternalInput").ap()
    aps["out"] = nc.dram_tensor("out", [NP, E], F32, kind="ExternalOutput").ap()

    with tile.TileContext(nc) as tc:
        for _ in range(loop_factor):
            _attn_kernel(tc, aps)
    nc.compile()
    _CACHE[key] = nc
    return nc


def _make_in_maps(inputs):
    bf = ml_dtypes.bfloat16
    x = np.asarray(inputs["x"], dtype=np.float32)
    attn_bias = np.asarray(inputs["attn_bias"], dtype=np.float32)
    pad_mask = np.asarray(inputs["pad_mask"]).astype(bool)
    # fuse padding mask into the bias (log-mask) and cast to bf16
    fused = attn_bias.astype(bf)
    fused[:, :, 1:, 1:] = np.where(
        pad_mask[:, 0:1, :, :], fused[:, :, 1:, 1:], bf(NEG)
    )
    ws = {w: np.asarray(inputs[w], dtype=np.float32) for w in ("Wq", "Wk", "Wv", "Wo")}
    bs = {b: np.asarray(inputs[b], dtype=np.float32) for b in ("bq", "bk", "bv", "bo")}
    in_maps = []
    for c in range(B):
        m = {
            "x": np.ascontiguousarray(x[c]),
            "fused": np.ascontiguousarray(fused[c]),
        }
        m.update(ws)
        m.update(bs)
        in_maps.append(m)
    return in_maps


def kernel(**inputs) -> np.ndarray:
    nc = _build()
    in_maps = _make_in_maps(inputs)
    res = run_bass_kernel_spmd(nc, in_maps, core_ids=list(range(B)))
    out = np.stack([res.results[c]["out"] for c in range(B)], axis=0)
    return out.astype(np.float32)


# revision 8
# speedup vs baseline: 2.9234x; 2.9234x over previous
"""Trainium2 Bass kernel for nn_GeneralAttn (multi-head attention with
structural attention bias + padding mask), data-parallel over batch B=8
across 8 NeuronCores.

v3 design (informed by HW microbenchmarks):
  - Host pre-fuses pad_mask into attn_bias (bias + log-mask) as FLOAT32:
    the DVE add (S_psum + bias -> bf16) is only fast when the SBUF operand
    is f32 (bf16 second operands hit a ~10x slow path on DVE).
  - All transposes ride the DMA XBAR (dma_start_transpose, 16-bit):
    x/W setup transposes, P -> P^T, and the per-pair attention-output
    transpose straight into the concat-head buffer catT (no PE transposes,
    no PSUM->SBUF copyback multiplies).
  - Matmuls in bf16 (f32 PSUM). Head pairs sit on partitions 0:64/64:128
    (disjoint PE row groups for S; per-pair at2/catT tiles).
  - Softmax normalization happens after PV on the tiny [q,64] output via
    the ones-column denominator (exp stays a single plain ACT pass).
  - Stages are emitted software-pipelined (skewed) because every engine
    executes its queue in order: bias +4 | S +2 | add,exp +1 | P-XBAR 0 |
    PV -2 | rcp,at -3 | at2-XBAR -4 | outproj -5 | store -6.
"""

import numpy as np
from contextlib import ExitStack

import ml_dtypes

import concourse.bass as bass
import concourse.bacc as bacc
import concourse.tile as tile
import concourse.mybir as mybir
from concourse.bass_utils import run_bass_kernel_spmd
from concourse._compat import with_exitstack

F32 = mybir.dt.float32
BF16 = mybir.dt.bfloat16
AF = mybir.ActivationFunctionType
OP = mybir.AluOpType

B = 8
NP = 1025
E = 512
H = 8
D = 64
N = NP - 1
NSUB = 9          # ceil(1025/128)
SEQ = NSUB * 128  # 1152
ESUB = 4          # 512/128
INV_SQRT_D = 1.0 / 8.0
NEG = -60000.0    # exp(NEG + bounded) == 0
SCHUNKS = ((0, 512), (512, 512), (1024, 128))


@with_exitstack
def _attn_kernel(ctx: ExitStack, tc: tile.TileContext, aps: dict):
    nc = tc.nc

    # ---------------- persistent buffers ----------------
    persist = ctx.enter_context(tc.tile_pool(name="persist", bufs=1))
    QT = persist.tile([128, ESUB, SEQ], BF16, tag="QT")       # [dq, dsub, s]
    KT = persist.tile([128, ESUB, SEQ], BF16, tag="KT")
    Vaug = persist.tile([128, NSUB, H, D + 1], BF16, tag="Vaug")
    catT = persist.tile([128, ESUB, SEQ], BF16, tag="catT")   # [hd, hsub, s]
    WTo = persist.tile([128, ESUB, ESUB, 128], BF16, tag="WTo")
    ones_bf = persist.tile([1, 128], BF16, tag="ones_bf")
    bo_row = persist.tile([1, E], BF16, tag="bo_row")
    bv_row = persist.tile([1, E], BF16, tag="bv_row")
    bqs = persist.tile([128, ESUB], F32, tag="bqs")
    bks = persist.tile([128, ESUB], F32, tag="bks")

    nc.gpsimd.memset(ones_bf[:], 1.0)

    # ---------------- setup (scoped pools; freed before main loop) -------
    with tc.tile_pool(name="setup", bufs=1) as setup, \
         tc.tile_pool(name="setup2", bufs=2) as setup2, \
         tc.tile_pool(name="pr_ps", bufs=2, space="PSUM") as pr_ps, \
         tc.tile_pool(name="pv_ps", bufs=2, space="PSUM") as pv_ps:

        # small vectors: bq scaled by 1/sqrt(D) (folded into the Q side)
        bsm = setup.tile([128, 2, ESUB], F32, tag="bsm")
        nc.sync.dma_start(out=bsm[:, 0], in_=aps["bq"].rearrange("(o p) -> p o", p=128))
        nc.sync.dma_start(out=bsm[:, 1], in_=aps["bk"].rearrange("(o p) -> p o", p=128))
        nc.vector.tensor_scalar(bqs[:], bsm[:, 0], INV_SQRT_D, None, OP.mult)
        nc.vector.tensor_copy(bks[:], bsm[:, 1])
        brow = setup.tile([1, 2, E], F32, tag="brow")
        nc.sync.dma_start(out=brow[:, 0], in_=aps["bo"].rearrange("(a e) -> a e", a=1))
        nc.sync.dma_start(out=brow[:, 1], in_=aps["bv"].rearrange("(a e) -> a e", a=1))
        nc.vector.tensor_copy(bo_row[:], brow[:, 0])
        nc.vector.tensor_copy(bv_row[:], brow[:, 1])

        # x: natural load, cast to bf16, XBAR-transpose per 128-row block
        xn = setup.tile([128, NSUB, E], F32, tag="xn")
        nc.vector.memset(xn[:, 8, :], 0.0)
        nc.sync.dma_start(
            out=xn[:, 0:8, :], in_=aps["x"][0:1024, :].rearrange("(o p) f -> p o f", p=128)
        )
        nc.sync.dma_start(out=xn[0:1, 8, :], in_=aps["x"][1024:1025, :].rearrange("a f -> a f"))
        xbf = setup.tile([128, NSUB, E], BF16, tag="xbf")
        nc.vector.tensor_copy(xbf[:], xn[:])
        # xT[:, s, e, :][p, c] = x^T[e*128+p, s*128+c]
        xT = setup.tile([128, NSUB, ESUB, 128], BF16, tag="xT")
        for s in range(NSUB):
            nc.scalar.dma_start_transpose(xT[:, s], xbf[:, s, :])

        # weights: natural load, cast (Wq scaled), XBAR-transpose.
        # WTx[:, o, e, :][p, c] = W^T[e*128+p, o*128+c]
        WTq = setup.tile([128, ESUB, ESUB, 128], BF16, tag="WTq")
        WTk = setup.tile([128, ESUB, ESUB, 128], BF16, tag="WTk")
        WTv = setup.tile([128, ESUB, ESUB, 128], BF16, tag="WTv")
        for wname, WTx, scale in (
            ("Wq", WTq, INV_SQRT_D), ("Wk", WTk, 1.0), ("Wv", WTv, 1.0), ("Wo", WTo, 1.0)
        ):
            wn = setup2.tile([128, ESUB, E], F32, tag="wn")
            nc.sync.dma_start(out=wn[:], in_=aps[wname].rearrange("(o p) f -> p o f", p=128))
            wb = setup2.tile([128, ESUB, E], BF16, tag="wb")
            nc.vector.tensor_scalar(wb[:], wn[:], scale, None, OP.mult)
            for o in range(ESUB):
                nc.scalar.dma_start_transpose(WTx[:, o], wb[:, o, :])

        # Q^T / K^T projections
        for WTx, QTx, bcol in ((WTq, QT, bqs), (WTk, KT, bks)):
            for dsub in range(ESUB):
                pr = pr_ps.tile([128, SEQ], F32, tag="pr")
                for e in range(ESUB):
                    for s0, sn in ((0, 4), (4, 4), (8, 1)):
                        nc.tensor.matmul(
                            pr[:, s0 * 128:(s0 + sn) * 128],
                            WTx[:, dsub, e, :],
                            xT[:, s0:s0 + sn, e, :],
                            start=(e == 0),
                            stop=(e == ESUB - 1),
                        )
                nc.vector.tensor_scalar(
                    QTx[:, dsub, :], pr[:], bcol[:, dsub:dsub + 1], None, OP.add
                )

        # V projection -> Vaug [k, kblk, h, 64|1]
        nc.vector.memset(Vaug[:, :, :, D:D + 1], 1.0)
        for s in range(NSUB):
            pv = pv_ps.tile([128, E], F32, tag="pv")
            nc.tensor.matmul(pv[:], ones_bf[:], bv_row[:], start=True, stop=False)
            for e in range(ESUB):
                nc.tensor.matmul(
                    pv[:],
                    xT[:, s, e, :],
                    WTv[:, 0:ESUB, e, :],
                    start=False,
                    stop=(e == ESUB - 1),
                    skip_group_check=True,
                )
            nc.vector.tensor_copy(
                Vaug[:, s, :, 0:D],
                pv[:].rearrange("p (h d) -> p h d", h=H),
            )

    # ---------------- main loop (software-pipelined emission) -------------
    with tc.tile_pool(name="bias_p", bufs=6) as bias_p, \
         tc.tile_pool(name="ssb_p", bufs=3) as ssb_p, \
         tc.tile_pool(name="p0_p", bufs=4) as p0_p, \
         tc.tile_pool(name="rc_p", bufs=4) as rc_p, \
         tc.tile_pool(name="pt_p", bufs=4) as pt_p, \
         tc.tile_pool(name="at2_p", bufs=3) as at2_p, \
         tc.tile_pool(name="osb_p", bufs=2) as osb_p, \
         tc.tile_pool(name="s_ps", bufs=2, space="PSUM") as s_ps, \
         tc.tile_pool(name="acc_ps", bufs=2, space="PSUM") as acc_ps:

        fused = aps["fused"]
        T = NSUB * H  # 72; t -> (qs, h), qs-major
        tiles = [dict() for _ in range(T)]

        def loc(t):
            qs, h = divmod(t, H)
            return qs, h, (128 if qs < 8 else 1), qs * 128, (h % 2) * 64, h // 2

        def em_bias(t):
            qs, h, rows, q0, hp0, hs = loc(t)
            bias_t = bias_p.tile([128, NP + 1], F32, tag="bias")
            nc.sync.dma_start(out=bias_t[0:rows, 0:NP], in_=fused[h, q0:q0 + rows, :])
            tiles[t]["bias"] = bias_t

        def em_s(t):
            qs, h, rows, q0, hp0, hs = loc(t)
            sp = s_ps.tile([128, SEQ], F32, tag="sps")
            qt = QT[hp0:hp0 + 64, hs, q0:q0 + 128]
            for c0, cm in SCHUNKS:
                nc.tensor.matmul(
                    sp[:, c0:c0 + cm], qt, KT[hp0:hp0 + 64, hs, c0:c0 + cm],
                    start=True, stop=True,
                )
            tiles[t]["sp"] = sp

        def em_add(t):
            sp = tiles[t].pop("sp")
            bias_t = tiles[t].pop("bias")
            s_sb = ssb_p.tile([128, SEQ], BF16, tag="ssb")
            nc.vector.tensor_tensor(s_sb[:, 0:NP], sp[:, 0:NP], bias_t[:, 0:NP], OP.add)
            tiles[t]["ssb"] = s_sb

        def em_exp(t):
            s_sb = tiles[t].pop("ssb")
            p0 = p0_p.tile([128, SEQ], BF16, tag="p0")
            nc.vector.memset(p0[:, NP:SEQ], 0.0)
            nc.scalar.activation(p0[:, 0:NP], s_sb[:, 0:NP], AF.Exp)
            tiles[t]["p0"] = p0

        def em_xbar(t):
            p0 = tiles[t].pop("p0")
            pt = pt_p.tile([128, NSUB, 128], BF16, tag="pt")
            nc.scalar.dma_start_transpose(pt[:], p0[:])
            tiles[t]["pt"] = pt

        def em_pv(t):
            qs, h, rows, q0, hp0, hs = loc(t)
            pt = tiles[t].pop("pt")
            pv = acc_ps.tile([128, E], F32, tag="acc")
            for j in range(NSUB):
                nc.tensor.matmul(
                    pv[:, 0:D + 1],
                    pt[:, j, :],
                    Vaug[:, j, h, :],
                    start=(j == 0),
                    stop=(j == NSUB - 1),
                )
            tiles[t]["pv"] = pv

        def em_at(t):
            qs, h, rows, q0, hp0, hs = loc(t)
            pv = tiles[t].pop("pv")
            qw = rows
            rc = rc_p.tile([128, 1], F32, tag="rc")
            nc.vector.reciprocal(rc[0:qw], pv[0:qw, D:D + 1])
            if h % 2 == 0:
                at2 = at2_p.tile([128, 128], BF16, tag="at2")
                tiles[t]["at2"] = at2
            else:
                at2 = tiles[t - 1]["at2"]
                tiles[t]["at2"] = at2
            nc.vector.tensor_scalar(
                at2[0:qw, hp0:hp0 + D], pv[0:qw, 0:D], rc[0:qw], None, OP.mult
            )

        def em_at2xbar(t):
            # t odd head of pair: transpose [q, 2x64] pair block into catT
            qs, h, rows, q0, hp0, hs = loc(t)
            at2 = tiles[t].pop("at2")
            tiles[t - 1].pop("at2", None)
            nc.scalar.dma_start_transpose(catT[:, hs, q0:q0 + 128], at2[:])

        def em_outproj(t):
            qs, h, rows, q0, hp0, hs = loc(t)
            op = acc_ps.tile([128, E], F32, tag="acc")
            nc.tensor.matmul(op[:], ones_bf[:], bo_row[:], start=True, stop=False)
            for hd in range(ESUB):
                nc.tensor.matmul(
                    op[:],
                    catT[:, hd, q0:q0 + 128],
                    WTo[:, 0:ESUB, hd, :],
                    start=False,
                    stop=(hd == ESUB - 1),
                    skip_group_check=True,
                )
            tiles[t]["op"] = op

        def em_out(t):
            qs, h, rows, q0, hp0, hs = loc(t)
            op = tiles[t].pop("op")
            o_sb = osb_p.tile([128, E], F32, tag="osb")
            nc.vector.tensor_copy(o_sb[0:rows, :], op[0:rows, :])
            nc.scalar.dma_start(out=aps["out"][q0:q0 + rows, :], in_=o_sb[0:rows, :])

        for t in range(T + 7):
            if t == 0:
                for u in range(min(4, T)):
                    em_bias(u)
                for u in range(min(2, T)):
                    em_s(u)
                em_add(0)
                em_exp(0)
            if t + 4 < T:
                em_bias(t + 4)
            if t + 2 < T:
                em_s(t + 2)
            if t + 1 < T:
                em_add(t + 1)
                em_exp(t + 1)
            if t < T:
                em_xbar(t)
            if 0 <= t - 2 < T:
                em_pv(t - 2)
            if 0 <= t - 3 < T:
                em_at(t - 3)
            if 0 <= t - 4 < T and (t - 4) % 2 == 1:
                em_at2xbar(t - 4)
            if 0 <= t - 5 < T and (t - 5) % H == H - 1:
                em_outproj(t - 5)
            if 0 <= t - 6 < T and (t - 6) % H == H - 1:
                em_out(t - 6)


_CACHE = {}


def _build(loop_factor: int = 1):
    key = ("nc", loop_factor)
    if key in _CACHE:
        return _CACHE[key]
    nc = bacc.Bacc("TRN2", num_devices=B)
    aps = {
        "x": nc.dram_tensor("x", [NP, E], F32, kind="ExternalInput").ap(),
        "fused": nc.dram_tensor("fused", [H, NP, NP], F32, kind="ExternalInput").ap(),
    }
    for wname in ("Wq", "Wk", "Wv", "Wo"):
        aps[wname] = nc.dram_tensor(wname, [E, E], F32, kind="ExternalInput").ap()
    for bname in ("bq", "bk", "bv", "bo"):
        aps[bname] = nc.dram_tensor(bname, [E], F32, kind="ExternalInput").ap()
    aps["out"] = nc.dram_tensor("out", [NP, E], F32, kind="ExternalOutput").ap()

    with tile.TileContext(nc) as tc:
        for _ in range(loop_factor):
            _attn_kernel(tc, aps)
    nc.compile()
    _CACHE[key] = nc
    return nc


def _make_in_maps(inputs):
    x = np.asarray(inputs["x"], dtype=np.float32)
    attn_bias = np.asarray(inputs["attn_bias"], dtype=np.float32)
    pad_mask = np.asarray(inputs["pad_mask"]).astype(bool)
    # fuse padding mask into the bias (log-mask), keep f32
    fused = attn_bias.copy()
    fused[:, :, 1:, 1:] = np.where(
        pad_mask[:, 0:1, :, :], fused[:, :, 1:, 1:], np.float32(NEG)
    )
    ws = {w: np.asarray(inputs[w], dtype=np.float32) for w in ("Wq", "Wk", "Wv", "Wo")}
    bs = {b: np.asarray(inputs[b], dtype=np.float32) for b in ("bq", "bk", "bv", "bo")}
    in_maps = []
    for c in range(B):
        m = {
            "x": np.ascontiguousarray(x[c]),
            "fused": np.ascontiguousarray(fused[c]),
        }
        m.update(ws)
        m.update(bs)
        in_maps.append(m)
    return in_maps


def kernel(**inputs) -> np.ndarray:
    nc = _build()
    in_maps = _make_in_maps(inputs)
    res = run_bass_kernel_spmd(nc, in_maps, core_ids=list(range(B)))
    out = np.stack([res.results[c]["out"] for c in range(B)], axis=0)
    return out.astype(np.float32)


# revision 9
# speedup vs baseline: 3.1401x; 1.0741x over previous
"""Trainium2 Bass kernel for nn_GeneralAttn (multi-head attention with
structural attention bias + padding mask), data-parallel over batch B=8
across 8 NeuronCores.

v3 design (informed by HW microbenchmarks):
  - Host pre-fuses pad_mask into attn_bias (bias + log-mask) as FLOAT32:
    the DVE add (S_psum + bias -> bf16) is only fast when the SBUF operand
    is f32 (bf16 second operands hit a ~10x slow path on DVE).
  - All transposes ride the DMA XBAR (dma_start_transpose, 16-bit):
    x/W setup transposes, P -> P^T, and the per-pair attention-output
    transpose straight into the concat-head buffer catT (no PE transposes,
    no PSUM->SBUF copyback multiplies).
  - Matmuls in bf16 (f32 PSUM). Head pairs sit on partitions 0:64/64:128
    (disjoint PE row groups for S; per-pair at2/catT tiles).
  - Softmax normalization happens after PV on the tiny [q,64] output via
    the ones-column denominator (exp stays a single plain ACT pass).
  - Stages are emitted software-pipelined (skewed) because every engine
    executes its queue in order: bias +4 | S +2 | add,exp +1 | P-XBAR 0 |
    PV -2 | rcp,at -3 | at2-XBAR -4 | outproj -5 | store -6.
"""

import numpy as np
from contextlib import ExitStack

import ml_dtypes

import concourse.bass as bass
import concourse.bacc as bacc
import concourse.tile as tile
import concourse.mybir as mybir
from concourse.bass_utils import run_bass_kernel_spmd
from concourse._compat import with_exitstack

F32 = mybir.dt.float32
BF16 = mybir.dt.bfloat16
AF = mybir.ActivationFunctionType
OP = mybir.AluOpType

B = 8
NP = 1025
E = 512
H = 8
D = 64
N = NP - 1
NSUB = 9          # ceil(1025/128)
SEQ = NSUB * 128  # 1152
ESUB = 4          # 512/128
INV_SQRT_D = 1.0 / 8.0
NEG = -60000.0    # exp(NEG + bounded) == 0
SCHUNKS = ((0, 512), (512, 512), (1024, 128))


@with_exitstack
def _attn_kernel(ctx: ExitStack, tc: tile.TileContext, aps: dict):
    nc = tc.nc

    # ---------------- persistent buffers ----------------
    persist = ctx.enter_context(tc.tile_pool(name="persist", bufs=1))
    QT = persist.tile([128, ESUB, SEQ], BF16, tag="QT")       # [dq, dsub, s]
    KT = persist.tile([128, ESUB, SEQ], BF16, tag="KT")
    Vaug = persist.tile([128, NSUB, H, D + 1], BF16, tag="Vaug")
    catT = persist.tile([128, ESUB, SEQ], BF16, tag="catT")   # [hd, hsub, s]
    WTo = persist.tile([128, ESUB, ESUB, 128], BF16, tag="WTo")
    ones_bf = persist.tile([1, 128], BF16, tag="ones_bf")
    bo_row = persist.tile([1, E], BF16, tag="bo_row")
    bv_row = persist.tile([1, E], BF16, tag="bv_row")
    bqs = persist.tile([128, ESUB], F32, tag="bqs")
    bks = persist.tile([128, ESUB], F32, tag="bks")

    nc.gpsimd.memset(ones_bf[:], 1.0)

    # ---------------- setup (scoped pools; freed before main loop) -------
    with tc.tile_pool(name="setup", bufs=1) as setup, \
         tc.tile_pool(name="setup2", bufs=2) as setup2, \
         tc.tile_pool(name="pr_ps", bufs=2, space="PSUM") as pr_ps, \
         tc.tile_pool(name="pv_ps", bufs=2, space="PSUM") as pv_ps:

        # small vectors: bq scaled by 1/sqrt(D) (folded into the Q side)
        bsm = setup.tile([128, 2, ESUB], F32, tag="bsm")
        nc.sync.dma_start(out=bsm[:, 0], in_=aps["bq"].rearrange("(o p) -> p o", p=128))
        nc.sync.dma_start(out=bsm[:, 1], in_=aps["bk"].rearrange("(o p) -> p o", p=128))
        nc.vector.tensor_scalar(bqs[:], bsm[:, 0], INV_SQRT_D, None, OP.mult)
        nc.vector.tensor_copy(bks[:], bsm[:, 1])
        brow = setup.tile([1, 2, E], F32, tag="brow")
        nc.sync.dma_start(out=brow[:, 0], in_=aps["bo"].rearrange("(a e) -> a e", a=1))
        nc.sync.dma_start(out=brow[:, 1], in_=aps["bv"].rearrange("(a e) -> a e", a=1))
        nc.vector.tensor_copy(bo_row[:], brow[:, 0])
        nc.vector.tensor_copy(bv_row[:], brow[:, 1])

        # x: natural load, cast to bf16, XBAR-transpose per 128-row block
        xn = setup.tile([128, NSUB, E], F32, tag="xn")
        nc.vector.memset(xn[:, 8, :], 0.0)
        nc.sync.dma_start(
            out=xn[:, 0:8, :], in_=aps["x"][0:1024, :].rearrange("(o p) f -> p o f", p=128)
        )
        nc.sync.dma_start(out=xn[0:1, 8, :], in_=aps["x"][1024:1025, :].rearrange("a f -> a f"))
        xbf = setup.tile([128, NSUB, E], BF16, tag="xbf")
        nc.vector.tensor_copy(xbf[:], xn[:])
        # xT[:, s, e, :][p, c] = x^T[e*128+p, s*128+c]
        xT = setup.tile([128, NSUB, ESUB, 128], BF16, tag="xT")
        for s in range(NSUB):
            nc.scalar.dma_start_transpose(xT[:, s], xbf[:, s, :])

        # weights: natural load, cast (Wq scaled), XBAR-transpose.
        # WTx[:, o, e, :][p, c] = W^T[e*128+p, o*128+c]
        WTq = setup.tile([128, ESUB, ESUB, 128], BF16, tag="WTq")
        WTk = setup.tile([128, ESUB, ESUB, 128], BF16, tag="WTk")
        WTv = setup.tile([128, ESUB, ESUB, 128], BF16, tag="WTv")
        for wname, WTx, scale in (
            ("Wq", WTq, INV_SQRT_D), ("Wk", WTk, 1.0), ("Wv", WTv, 1.0), ("Wo", WTo, 1.0)
        ):
            wn = setup2.tile([128, ESUB, E], F32, tag="wn")
            nc.sync.dma_start(out=wn[:], in_=aps[wname].rearrange("(o p) f -> p o f", p=128))
            wb = setup2.tile([128, ESUB, E], BF16, tag="wb")
            nc.vector.tensor_scalar(wb[:], wn[:], scale, None, OP.mult)
            for o in range(ESUB):
                nc.scalar.dma_start_transpose(WTx[:, o], wb[:, o, :])

        # Q^T / K^T projections
        for WTx, QTx, bcol in ((WTq, QT, bqs), (WTk, KT, bks)):
            for dsub in range(ESUB):
                pr = pr_ps.tile([128, SEQ], F32, tag="pr")
                for e in range(ESUB):
                    for s0, sn in ((0, 4), (4, 4), (8, 1)):
                        nc.tensor.matmul(
                            pr[:, s0 * 128:(s0 + sn) * 128],
                            WTx[:, dsub, e, :],
                            xT[:, s0:s0 + sn, e, :],
                            start=(e == 0),
                            stop=(e == ESUB - 1),
                        )
                nc.vector.tensor_scalar(
                    QTx[:, dsub, :], pr[:], bcol[:, dsub:dsub + 1], None, OP.add
                )

        # V projection -> Vaug [k, kblk, h, 64|1]
        nc.vector.memset(Vaug[:, :, :, D:D + 1], 1.0)
        for s in range(NSUB):
            pv = pv_ps.tile([128, E], F32, tag="pv")
            nc.tensor.matmul(pv[:], ones_bf[:], bv_row[:], start=True, stop=False)
            for e in range(ESUB):
                nc.tensor.matmul(
                    pv[:],
                    xT[:, s, e, :],
                    WTv[:, 0:ESUB, e, :],
                    start=False,
                    stop=(e == ESUB - 1),
                    skip_group_check=True,
                )
            nc.vector.tensor_copy(
                Vaug[:, s, :, 0:D],
                pv[:].rearrange("p (h d) -> p h d", h=H),
            )

    # ---------------- main loop (software-pipelined emission) -------------
    with tc.tile_pool(name="bias_p", bufs=6) as bias_p, \
         tc.tile_pool(name="ssb_p", bufs=3) as ssb_p, \
         tc.tile_pool(name="p0_p", bufs=4) as p0_p, \
         tc.tile_pool(name="rc_p", bufs=4) as rc_p, \
         tc.tile_pool(name="pt_p", bufs=4) as pt_p, \
         tc.tile_pool(name="at2_p", bufs=3) as at2_p, \
         tc.tile_pool(name="osb_p", bufs=2) as osb_p, \
         tc.tile_pool(name="s_ps", bufs=2, space="PSUM") as s_ps, \
         tc.tile_pool(name="acc_ps", bufs=2, space="PSUM") as acc_ps:

        fused = aps["fused"]
        T = NSUB * H  # 72; t -> (qs, h), qs-major
        tiles = [dict() for _ in range(T)]

        def loc(t):
            qs, h = divmod(t, H)
            return qs, h, (128 if qs < 8 else 1), qs * 128, (h % 2) * 64, h // 2

        def em_bias(t):
            qs, h, rows, q0, hp0, hs = loc(t)
            bias_t = bias_p.tile([128, NP + 1], F32, tag="bias")
            nc.sync.dma_start(out=bias_t[0:rows, 0:NP], in_=fused[h, q0:q0 + rows, :])
            tiles[t]["bias"] = bias_t

        def em_s(t):
            qs, h, rows, q0, hp0, hs = loc(t)
            sp = s_ps.tile([128, SEQ], F32, tag="sps")
            qt = QT[hp0:hp0 + 64, hs, q0:q0 + 128]
            for c0, cm in SCHUNKS:
                nc.tensor.matmul(
                    sp[:, c0:c0 + cm], qt, KT[hp0:hp0 + 64, hs, c0:c0 + cm],
                    start=True, stop=True,
                )
            tiles[t]["sp"] = sp

        def em_add(t):
            sp = tiles[t].pop("sp")
            bias_t = tiles[t].pop("bias")
            s_sb = ssb_p.tile([128, SEQ], F32, tag="ssb")
            nc.vector.tensor_tensor(s_sb[:, 0:NP], sp[:, 0:NP], bias_t[:, 0:NP], OP.add)
            tiles[t]["ssb"] = s_sb

        def em_exp(t):
            s_sb = tiles[t].pop("ssb")
            p0 = p0_p.tile([128, SEQ], BF16, tag="p0")
            nc.vector.memset(p0[:, NP:SEQ], 0.0)
            nc.scalar.activation(p0[:, 0:NP], s_sb[:, 0:NP], AF.Exp)
            tiles[t]["p0"] = p0

        def em_xbar(t):
            p0 = tiles[t].pop("p0")
            pt = pt_p.tile([128, NSUB, 128], BF16, tag="pt")
            nc.scalar.dma_start_transpose(pt[:], p0[:])
            tiles[t]["pt"] = pt

        def em_pv(t):
            qs, h, rows, q0, hp0, hs = loc(t)
            pt = tiles[t].pop("pt")
            pv = acc_ps.tile([128, E], F32, tag="acc")
            for j in range(NSUB):
                nc.tensor.matmul(
                    pv[:, 0:D + 1],
                    pt[:, j, :],
                    Vaug[:, j, h, :],
                    start=(j == 0),
                    stop=(j == NSUB - 1),
                )
            tiles[t]["pv"] = pv

        def em_at(t):
            qs, h, rows, q0, hp0, hs = loc(t)
            pv = tiles[t].pop("pv")
            qw = rows
            rc = rc_p.tile([128, 1], F32, tag="rc")
            nc.vector.reciprocal(rc[0:qw], pv[0:qw, D:D + 1])
            if h % 2 == 0:
                at2 = at2_p.tile([128, 128], BF16, tag="at2")
                tiles[t]["at2"] = at2
            else:
                at2 = tiles[t - 1]["at2"]
                tiles[t]["at2"] = at2
            nc.vector.tensor_scalar(
                at2[0:qw, hp0:hp0 + D], pv[0:qw, 0:D], rc[0:qw], None, OP.mult
            )

        def em_at2xbar(t):
            # t odd head of pair: transpose [q, 2x64] pair block into catT
            qs, h, rows, q0, hp0, hs = loc(t)
            at2 = tiles[t].pop("at2")
            tiles[t - 1].pop("at2", None)
            nc.scalar.dma_start_transpose(catT[:, hs, q0:q0 + 128], at2[:])

        def em_outproj(t):
            qs, h, rows, q0, hp0, hs = loc(t)
            op = acc_ps.tile([128, E], F32, tag="acc")
            nc.tensor.matmul(op[:], ones_bf[:], bo_row[:], start=True, stop=False)
            for hd in range(ESUB):
                nc.tensor.matmul(
                    op[:],
                    catT[:, hd, q0:q0 + 128],
                    WTo[:, 0:ESUB, hd, :],
                    start=False,
                    stop=(hd == ESUB - 1),
                    skip_group_check=True,
                )
            tiles[t]["op"] = op

        def em_out(t):
            qs, h, rows, q0, hp0, hs = loc(t)
            op = tiles[t].pop("op")
            o_sb = osb_p.tile([128, E], F32, tag="osb")
            nc.vector.tensor_copy(o_sb[0:rows, :], op[0:rows, :])
            nc.scalar.dma_start(out=aps["out"][q0:q0 + rows, :], in_=o_sb[0:rows, :])

        for t in range(T + 7):
            if t == 0:
                for u in range(min(4, T)):
                    em_bias(u)
                for u in range(min(2, T)):
                    em_s(u)
                em_add(0)
                em_exp(0)
            if t + 4 < T:
                em_bias(t + 4)
            if t + 2 < T:
                em_s(t + 2)
            if t + 1 < T:
                em_add(t + 1)
                em_exp(t + 1)
            if t < T:
                em_xbar(t)
            if 0 <= t - 2 < T:
                em_pv(t - 2)
            if 0 <= t - 3 < T:
                em_at(t - 3)
            if 0 <= t - 4 < T and (t - 4) % 2 == 1:
                em_at2xbar(t - 4)
            if 0 <= t - 5 < T and (t - 5) % H == H - 1:
                em_outproj(t - 5)
            if 0 <= t - 6 < T and (t - 6) % H == H - 1:
                em_out(t - 6)


_CACHE = {}


def _build(loop_factor: int = 1):
    key = ("nc", loop_factor)
    if key in _CACHE:
        return _CACHE[key]
    nc = bacc.Bacc("TRN2", num_devices=B)
    aps = {
        "x": nc.dram_tensor("x", [NP, E], F32, kind="ExternalInput").ap(),
        "fused": nc.dram_tensor("fused", [H, NP, NP], F32, kind="ExternalInput").ap(),
    }
    for wname in ("Wq", "Wk", "Wv", "Wo"):
        aps[wname] = nc.dram_tensor(wname, [E, E], F32, kind="ExternalInput").ap()
    for bname in ("bq", "bk", "bv", "bo"):
        aps[bname] = nc.dram_tensor(bname, [E], F32, kind="ExternalInput").ap()
    aps["out"] = nc.dram_tensor("out", [NP, E], F32, kind="ExternalOutput").ap()

    with tile.TileContext(nc) as tc:
        for _ in range(loop_factor):
            _attn_kernel(tc, aps)
    nc.compile()
    _CACHE[key] = nc
    return nc


def _make_in_maps(inputs):
    x = np.asarray(inputs["x"], dtype=np.float32)
    attn_bias = np.asarray(inputs["attn_bias"], dtype=np.float32)
    pad_mask = np.asarray(inputs["pad_mask"]).astype(bool)
    # fuse padding mask into the bias (log-mask), keep f32
    fused = attn_bias.copy()
    fused[:, :, 1:, 1:] = np.where(
        pad_mask[:, 0:1, :, :], fused[:, :, 1:, 1:], np.float32(NEG)
    )
    ws = {w: np.asarray(inputs[w], dtype=np.float32) for w in ("Wq", "Wk", "Wv", "Wo")}
    bs = {b: np.asarray(inputs[b], dtype=np.float32) for b in ("bq", "bk", "bv", "bo")}
    in_maps = []
    for c in range(B):
        m = {
            "x": np.ascontiguousarray(x[c]),
            "fused": np.ascontiguousarray(fused[c]),
        }
        m.update(ws)
        m.update(bs)
        in_maps.append(m)
    return in_maps


def kernel(**inputs) -> np.ndarray:
    nc = _build()
    in_maps = _make_in_maps(inputs)
    res = run_bass_kernel_spmd(nc, in_maps, core_ids=list(range(B)))
    out = np.stack([res.results[c]["out"] for c in range(B)], axis=0)
    return out.astype(np.float32)


# revision 11
# speedup vs baseline: 11.0305x; 3.5128x over previous
"""Trainium2 Bass kernel for nn_GeneralAttn (multi-head attention with
structural attention bias + padding mask), data-parallel over batch B=8
across 8 NeuronCores.

v3 design (informed by HW microbenchmarks):
  - Host pre-fuses pad_mask into attn_bias (bias + log-mask) as FLOAT32:
    the DVE add (S_psum + bias -> bf16) is only fast when the SBUF operand
    is f32 (bf16 second operands hit a ~10x slow path on DVE).
  - All transposes ride the DMA XBAR (dma_start_transpose, 16-bit):
    x/W setup transposes, P -> P^T, and the per-pair attention-output
    transpose straight into the concat-head buffer catT (no PE transposes,
    no PSUM->SBUF copyback multiplies).
  - Matmuls in bf16 (f32 PSUM). Head pairs sit on partitions 0:64/64:128
    (disjoint PE row groups for S; per-pair at2/catT tiles).
  - Softmax normalization happens after PV on the tiny [q,64] output via
    the ones-column denominator (exp stays a single plain ACT pass).
  - Stages are emitted software-pipelined (skewed) because every engine
    executes its queue in order: bias +4 | S +2 | add,exp +1 | P-XBAR 0 |
    PV -2 | rcp,at -3 | at2-XBAR -4 | outproj -5 | store -6.
"""

import numpy as np
from contextlib import ExitStack

import ml_dtypes

import concourse.bass as bass
import concourse.bacc as bacc
import concourse.tile as tile
import concourse.mybir as mybir
from concourse.bass_utils import run_bass_kernel_spmd
from concourse._compat import with_exitstack

F32 = mybir.dt.float32
BF16 = mybir.dt.bfloat16
AF = mybir.ActivationFunctionType
OP = mybir.AluOpType

B = 8
NP = 1025
E = 512
H = 8
D = 64
N = NP - 1
NSUB = 9          # ceil(1025/128)
SEQ = NSUB * 128  # 1152
ESUB = 4          # 512/128
INV_SQRT_D = 1.0 / 8.0
NEG = -60000.0    # exp(NEG + bounded) == 0
SCHUNKS = ((0, 512), (512, 512), (1024, 128))


@with_exitstack
def _attn_kernel(ctx: ExitStack, tc: tile.TileContext, aps: dict):
    nc = tc.nc

    # ---------------- persistent buffers ----------------
    persist = ctx.enter_context(tc.tile_pool(name="persist", bufs=1))
    QT = persist.tile([128, ESUB, SEQ], BF16, tag="QT")       # [dq, dsub, s]
    KT = persist.tile([128, ESUB, SEQ], BF16, tag="KT")
    Vaug = persist.tile([128, NSUB, H, D + 1], BF16, tag="Vaug")
    catT = persist.tile([128, ESUB, SEQ], BF16, tag="catT")   # [hd, hsub, s]
    WTo = persist.tile([128, ESUB, ESUB, 128], BF16, tag="WTo")
    ones_bf = persist.tile([1, 128], BF16, tag="ones_bf")
    bo_row = persist.tile([1, E], BF16, tag="bo_row")
    bv_row = persist.tile([1, E], BF16, tag="bv_row")
    bqs = persist.tile([128, ESUB], F32, tag="bqs")
    bks = persist.tile([128, ESUB], F32, tag="bks")

    nc.gpsimd.memset(ones_bf[:], 1.0)

    # ---------------- setup (scoped pools; freed before main loop) -------
    with tc.tile_pool(name="setup", bufs=1) as setup, \
         tc.tile_pool(name="setup2", bufs=2) as setup2, \
         tc.tile_pool(name="pr_ps", bufs=2, space="PSUM") as pr_ps, \
         tc.tile_pool(name="pv_ps", bufs=2, space="PSUM") as pv_ps:

        # small vectors: bq scaled by 1/sqrt(D) (folded into the Q side)
        bsm = setup.tile([128, 2, ESUB], F32, tag="bsm")
        nc.sync.dma_start(out=bsm[:, 0], in_=aps["bq"].rearrange("(o p) -> p o", p=128))
        nc.sync.dma_start(out=bsm[:, 1], in_=aps["bk"].rearrange("(o p) -> p o", p=128))
        nc.vector.tensor_scalar(bqs[:], bsm[:, 0], INV_SQRT_D, None, OP.mult)
        nc.vector.tensor_copy(bks[:], bsm[:, 1])
        brow = setup.tile([1, 2, E], F32, tag="brow")
        nc.sync.dma_start(out=brow[:, 0], in_=aps["bo"].rearrange("(a e) -> a e", a=1))
        nc.sync.dma_start(out=brow[:, 1], in_=aps["bv"].rearrange("(a e) -> a e", a=1))
        nc.vector.tensor_copy(bo_row[:], brow[:, 0])
        nc.vector.tensor_copy(bv_row[:], brow[:, 1])

        # x: natural load, cast to bf16, XBAR-transpose per 128-row block
        xn = setup.tile([128, NSUB, E], F32, tag="xn")
        nc.vector.memset(xn[:, 8, :], 0.0)
        nc.sync.dma_start(
            out=xn[:, 0:8, :], in_=aps["x"][0:1024, :].rearrange("(o p) f -> p o f", p=128)
        )
        nc.sync.dma_start(out=xn[0:1, 8, :], in_=aps["x"][1024:1025, :].rearrange("a f -> a f"))
        xbf = setup.tile([128, NSUB, E], BF16, tag="xbf")
        nc.vector.tensor_copy(xbf[:], xn[:])
        # xT[:, s, e, :][p, c] = x^T[e*128+p, s*128+c]
        xT = setup.tile([128, NSUB, ESUB, 128], BF16, tag="xT")
        for s in range(NSUB):
            nc.scalar.dma_start_transpose(xT[:, s], xbf[:, s, :])

        # weights: natural load, cast (Wq scaled), XBAR-transpose.
        # WTx[:, o, e, :][p, c] = W^T[e*128+p, o*128+c]
        WTq = setup.tile([128, ESUB, ESUB, 128], BF16, tag="WTq")
        WTk = setup.tile([128, ESUB, ESUB, 128], BF16, tag="WTk")
        WTv = setup.tile([128, ESUB, ESUB, 128], BF16, tag="WTv")
        for wname, WTx, scale in (
            ("Wq", WTq, INV_SQRT_D), ("Wk", WTk, 1.0), ("Wv", WTv, 1.0), ("Wo", WTo, 1.0)
        ):
            wn = setup2.tile([128, ESUB, E], F32, tag="wn")
            nc.sync.dma_start(out=wn[:], in_=aps[wname].rearrange("(o p) f -> p o f", p=128))
            wb = setup2.tile([128, ESUB, E], BF16, tag="wb")
            nc.vector.tensor_scalar(wb[:], wn[:], scale, None, OP.mult)
            for o in range(ESUB):
                nc.scalar.dma_start_transpose(WTx[:, o], wb[:, o, :])

        # Q^T / K^T projections
        for WTx, QTx, bcol in ((WTq, QT, bqs), (WTk, KT, bks)):
            for dsub in range(ESUB):
                pr = pr_ps.tile([128, SEQ], F32, tag="pr")
                for e in range(ESUB):
                    for s0, sn in ((0, 4), (4, 4), (8, 1)):
                        nc.tensor.matmul(
                            pr[:, s0 * 128:(s0 + sn) * 128],
                            WTx[:, dsub, e, :],
                            xT[:, s0:s0 + sn, e, :],
                            start=(e == 0),
                            stop=(e == ESUB - 1),
                        )
                nc.vector.tensor_scalar(
                    QTx[:, dsub, :], pr[:], bcol[:, dsub:dsub + 1], None, OP.add
                )

        # V projection -> Vaug [k, kblk, h, 64|1]
        nc.vector.memset(Vaug[:, :, :, D:D + 1], 1.0)
        for s in range(NSUB):
            pv = pv_ps.tile([128, E], F32, tag="pv")
            nc.tensor.matmul(pv[:], ones_bf[:], bv_row[:], start=True, stop=False)
            for e in range(ESUB):
                nc.tensor.matmul(
                    pv[:],
                    xT[:, s, e, :],
                    WTv[:, 0:ESUB, e, :],
                    start=False,
                    stop=(e == ESUB - 1),
                    skip_group_check=True,
                )
            nc.vector.tensor_copy(
                Vaug[:, s, :, 0:D],
                pv[:].rearrange("p (h d) -> p h d", h=H),
            )

    # ---------------- main loop (software-pipelined emission) -------------
    with tc.tile_pool(name="bias_p", bufs=6) as bias_p, \
         tc.tile_pool(name="ssb_p", bufs=3) as ssb_p, \
         tc.tile_pool(name="p0_p", bufs=4) as p0_p, \
         tc.tile_pool(name="rc_p", bufs=4) as rc_p, \
         tc.tile_pool(name="pt_p", bufs=4) as pt_p, \
         tc.tile_pool(name="at2_p", bufs=3) as at2_p, \
         tc.tile_pool(name="osb_p", bufs=2) as osb_p, \
         tc.tile_pool(name="s_ps", bufs=2, space="PSUM") as s_ps, \
         tc.tile_pool(name="acc_ps", bufs=2, space="PSUM") as acc_ps:

        fused = aps["fused"]
        T = NSUB * H  # 72; t -> (qs, h), qs-major
        tiles = [dict() for _ in range(T)]
        import os as _os
        if _os.environ.get("ABLATE_XBAR"):
            pt_dummy = pt_p.tile([128, NSUB, 128], BF16, tag="ptd")
            nc.gpsimd.memset(pt_dummy[:], 0.001)
            dbg_sink = pt_p.tile([1, 64], F32, tag="sink")

        def loc(t):
            qs, h = divmod(t, H)
            return qs, h, (128 if qs < 8 else 1), qs * 128, (h % 2) * 64, h // 2

        def em_bias(t):
            qs, h, rows, q0, hp0, hs = loc(t)
            bias_t = bias_p.tile([128, NP + 1], F32, tag="bias")
            nc.sync.dma_start(out=bias_t[0:rows, 0:NP], in_=fused[h, q0:q0 + rows, :])
            tiles[t]["bias"] = bias_t

        def em_s(t):
            qs, h, rows, q0, hp0, hs = loc(t)
            sp = s_ps.tile([128, SEQ], F32, tag="sps")
            qt = QT[hp0:hp0 + 64, hs, q0:q0 + 128]
            for c0, cm in SCHUNKS:
                nc.tensor.matmul(
                    sp[:, c0:c0 + cm], qt, KT[hp0:hp0 + 64, hs, c0:c0 + cm],
                    start=True, stop=True,
                )
            tiles[t]["sp"] = sp

        def em_add(t):
            sp = tiles[t].pop("sp")
            bias_t = tiles[t].pop("bias")
            s_sb = ssb_p.tile([128, SEQ], F32, tag="ssb")
            nc.vector.tensor_tensor(s_sb[:, 0:NP], sp[:, 0:NP], bias_t[:, 0:NP], OP.add)
            tiles[t]["ssb"] = s_sb

        def em_exp(t):
            s_sb = tiles[t].pop("ssb")
            p0 = p0_p.tile([128, SEQ], BF16, tag="p0")
            nc.vector.memset(p0[:, NP:SEQ], 0.0)
            nc.scalar.activation(p0[:, 0:NP], s_sb[:, 0:NP], AF.Exp)
            tiles[t]["p0"] = p0

        def em_xbar(t):
            import os
            p0 = tiles[t].pop("p0")
            if os.environ.get("ABLATE_XBAR"):
                # timing ablation: skip the transpose, keep p0 live via a
                # tiny read, feed PV a persistent dummy tile
                nc.vector.tensor_copy(dbg_sink[0:1, t % 64:t % 64 + 1], p0[0:1, 0:1])
                tiles[t]["pt"] = pt_dummy
                return
            pt = pt_p.tile([128, NSUB, 128], BF16, tag="pt")
            nc.scalar.dma_start_transpose(pt[:], p0[:])
            tiles[t]["pt"] = pt

        def em_pv(t):
            qs, h, rows, q0, hp0, hs = loc(t)
            pt = tiles[t].pop("pt")
            pv = acc_ps.tile([128, E], F32, tag="acc")
            for j in range(NSUB):
                nc.tensor.matmul(
                    pv[:, 0:D + 1],
                    pt[:, j, :],
                    Vaug[:, j, h, :],
                    start=(j == 0),
                    stop=(j == NSUB - 1),
                )
            tiles[t]["pv"] = pv

        def em_at(t):
            qs, h, rows, q0, hp0, hs = loc(t)
            pv = tiles[t].pop("pv")
            qw = rows
            rc = rc_p.tile([128, 1], F32, tag="rc")
            nc.vector.reciprocal(rc[0:qw], pv[0:qw, D:D + 1])
            if h % 2 == 0:
                at2 = at2_p.tile([128, 128], BF16, tag="at2")
                tiles[t]["at2"] = at2
            else:
                at2 = tiles[t - 1]["at2"]
                tiles[t]["at2"] = at2
            nc.vector.tensor_scalar(
                at2[0:qw, hp0:hp0 + D], pv[0:qw, 0:D], rc[0:qw], None, OP.mult
            )

        def em_at2xbar(t):
            # t odd head of pair: transpose [q, 2x64] pair block into catT
            qs, h, rows, q0, hp0, hs = loc(t)
            at2 = tiles[t].pop("at2")
            tiles[t - 1].pop("at2", None)
            nc.scalar.dma_start_transpose(catT[:, hs, q0:q0 + 128], at2[:])

        def em_outproj(t):
            qs, h, rows, q0, hp0, hs = loc(t)
            op = acc_ps.tile([128, E], F32, tag="acc")
            nc.tensor.matmul(op[:], ones_bf[:], bo_row[:], start=True, stop=False)
            for hd in range(ESUB):
                nc.tensor.matmul(
                    op[:],
                    catT[:, hd, q0:q0 + 128],
                    WTo[:, 0:ESUB, hd, :],
                    start=False,
                    stop=(hd == ESUB - 1),
                    skip_group_check=True,
                )
            tiles[t]["op"] = op

        def em_out(t):
            qs, h, rows, q0, hp0, hs = loc(t)
            op = tiles[t].pop("op")
            o_sb = osb_p.tile([128, E], F32, tag="osb")
            nc.vector.tensor_copy(o_sb[0:rows, :], op[0:rows, :])
            nc.scalar.dma_start(out=aps["out"][q0:q0 + rows, :], in_=o_sb[0:rows, :])

        for t in range(T + 7):
            if t == 0:
                for u in range(min(4, T)):
                    em_bias(u)
                for u in range(min(2, T)):
                    em_s(u)
                em_add(0)
                em_exp(0)
            if t + 4 < T:
                em_bias(t + 4)
            if t + 2 < T:
                em_s(t + 2)
            if t + 1 < T:
                em_add(t + 1)
                em_exp(t + 1)
            if t < T:
                em_xbar(t)
            if 0 <= t - 2 < T:
                em_pv(t - 2)
            if 0 <= t - 3 < T:
                em_at(t - 3)
            if 0 <= t - 4 < T and (t - 4) % 2 == 1:
                em_at2xbar(t - 4)
            if 0 <= t - 5 < T and (t - 5) % H == H - 1:
                em_outproj(t - 5)
            if 0 <= t - 6 < T and (t - 6) % H == H - 1:
                em_out(t - 6)


_CACHE = {}


def _build(loop_factor: int = 1):
    key = ("nc", loop_factor)
    if key in _CACHE:
        return _CACHE[key]
    nc = bacc.Bacc("TRN2", num_devices=B)
    aps = {
        "x": nc.dram_tensor("x", [NP, E], F32, kind="ExternalInput").ap(),
        "fused": nc.dram_tensor("fused", [H, NP, NP], F32, kind="ExternalInput").ap(),
    }
    for wname in ("Wq", "Wk", "Wv", "Wo"):
        aps[wname] = nc.dram_tensor(wname, [E, E], F32, kind="ExternalInput").ap()
    for bname in ("bq", "bk", "bv", "bo"):
        aps[bname] = nc.dram_tensor(bname, [E], F32, kind="ExternalInput").ap()
    aps["out"] = nc.dram_tensor("out", [NP, E], F32, kind="ExternalOutput").ap()

    with tile.TileContext(nc) as tc:
        for _ in range(loop_factor):
            _attn_kernel(tc, aps)
    nc.compile()
    _CACHE[key] = nc
    return nc


def _make_in_maps(inputs):
    x = np.asarray(inputs["x"], dtype=np.float32)
    attn_bias = np.asarray(inputs["attn_bias"], dtype=np.float32)
    pad_mask = np.asarray(inputs["pad_mask"]).astype(bool)
    # fuse padding mask into the bias (log-mask), keep f32
    fused = attn_bias.copy()
    fused[:, :, 1:, 1:] = np.where(
        pad_mask[:, 0:1, :, :], fused[:, :, 1:, 1:], np.float32(NEG)
    )
    ws = {w: np.asarray(inputs[w], dtype=np.float32) for w in ("Wq", "Wk", "Wv", "Wo")}
    bs = {b: np.asarray(inputs[b], dtype=np.float32) for b in ("bq", "bk", "bv", "bo")}
    in_maps = []
    for c in range(B):
        m = {
            "x": np.ascontiguousarray(x[c]),
            "fused": np.ascontiguousarray(fused[c]),
        }
        m.update(ws)
        m.update(bs)
        in_maps.append(m)
    return in_maps


def kernel(**inputs) -> np.ndarray:
    nc = _build()
    in_maps = _make_in_maps(inputs)
    res = run_bass_kernel_spmd(nc, in_maps, core_ids=list(range(B)))
    out = np.stack([res.results[c]["out"] for c in range(B)], axis=0)
    return out.astype(np.float32)
